# revision 53
# baseline (speedup 1.0000x reference)
"""Trainium2 Bass kernel for nn_MetricLoss (lifted-structure-style metric loss).

Reference computation (N=4096 rows, F=512 features, 16 label classes):
    Dsq = ||b_i||^2 + ||a_j||^2 - 2 b@a.T ;  D = sqrt(max(Dsq,0))   [N,N]
    Dexpm = exp(1 - D)
    row_negsum[i] = sum_{j: lbl_j != lbl_i} Dexpm[i,j]
    J = log(row_negsum[i] + row_negsum[j]) + D
    loss = sum_{i!=j, lbl_i==lbl_j} relu(J)^2 / (2 * num_pos)

Design — fully decoupled cores (NO collectives), measured 73.8us max-core
vs the v1 collective baseline's 123-210us (v1's per-core time included
multi-core launch-skew waits at its AllGather; here cores never wait on
each other, so per-core exec time is just own work and run-to-run
variance collapses to <2us):
  * Rows are sorted by label on the host; label classes are PAIRED
    (2-opt search minimizing (max pair rows, max window tiles)) and each
    core owns ALL rows of its 2 classes, padded with zero-rows to a
    common R_pad (544 here). Every positive pair (i, j) then has both
    ns_i and ns_j computed locally - the AllGather of row_negsum is gone.
  * Per-core column permutation sigma puts the core's window j-tiles
    (tiles overlapping its 2 classes) at slots 0..nt2-1, so the phase-2
    loop structure is core-independent (pure SPMD); all class masks are
    input data, not program structure.
  * The -2*b@a.T GEMM runs in fp8e4m3 with DoubleRow perf mode (0.5
    cycles/row, 2 matmuls per slot-chunk contracting 256 k each); the
    fp8 cross term perturbs the final loss by ~1.2e-4 rel (validated
    offline against the fp64 reference). Norm terms stay fp32-exact:
    aa[j] rides the DVE add as a per-partition scalar, bb[i] as a
    broadcast tile built on-device by an fp32 ones-matmul (keeps 278KB
    off the startup DMA window). `at` is fully SBUF-resident (fp8 =
    16KB/partition), DMA'd in 4 column splits so slot 0 lands early.
  * negsum via 3-column one-hot matmuls per j-tile (ones/classP/classQ);
    ns = total - own-class, combined with a [3,1] +-1 bf16 matmul.
  * ns_j in partition layout (nsT[128, slot]) is built with 5 tiny
    transpose matmuls + per-128-chunk (TI mask x PM permutation-matmul)
    accumulation; a trailing identity-block matmul folds in the 1.0
    fill for alien partitions. All SPMD-safe despite per-core offsets.
  * D_ii (diagonal) is host-precomputed (same O(N F) class as aa/bb).
  * ACT (the critical engine, ~44us busy: sqrt+exp over 2.2M elems plus
    phase-2 Ln) runs in 4 table blocks - sqrt chunks {1,1,2,4}, exp
    those, sqrt the rest (stt-cadence-gated), exp the rest - with the
    last exp split in half so its negsum matmuls gate ns ~1us less.
    Tiny first chunks start the sqrt stream ~1.5us earlier. ln(2 ns)
    for the diagonal fills the ACT idle gap while the nsT/ns-broadcast
    machinery runs; phase-2 Ln reads the ns broadcast straight from
    PSUM (only the bias requires SBUF). Exp/Ln share one table set
    (natural_log_exp_and_others pinned), so 3 loads total after the
    preamble sqrt load.
  * Phase-2 hinge-by-class matmuls accumulate into the drained negsum
    PSUM banks, keeping total PSUM use at exactly 8 banks with no
    serialization on pool space.
  * GEMM free dim is chunked 2x272 (R_pad=544) so matmul outputs stay
    within PSUM banks.
"""

import re
import operator
import numpy as np
import ml_dtypes
from contextlib import ExitStack

import concourse.bass as bass
import concourse.tile as tile
from concourse import bacc, mybir
from concourse import dve_ops
from concourse.dve_spec import Spec, Src0, Src1, C0, relu, sq
from concourse.bass_utils import run_bass_kernel_spmd
from concourse.tile_rust import add_dep_helper

F32 = mybir.dt.float32
BF16 = mybir.dt.bfloat16
FP8 = mybir.dt.float8e4
NPBF16 = ml_dtypes.bfloat16
NPFP8 = mybir.dt.np(mybir.dt.float8e4)
AF = mybir.ActivationFunctionType
ALU = mybir.AluOpType
DR = mybir.MatmulPerfMode.DoubleRow

N = 4096          # rows (a and b)
F = 512           # features
NCORES = 8
NT = N // 128     # j-tiles of 128 partitions = 32
NCLS = 16         # label classes
MARGIN = 1.0


def _register_sqrelu_add():
    """Custom fused DVE op: out = relu(in0 + in1)^2, accum_out = c0 + sum(out)."""
    name = "SQRELU_ADD_ANT"
    for op in dve_ops.OPS:
        if op.name == name:
            return op
    op = dve_ops.DveOp(
        name,
        Spec(body=sq(relu(Src0 + Src1)), accum=operator.add, accum_init=C0),
        subdim=False,
        uops_sha={},
    )
    dve_ops._SUB_OPCODE_FOR_NAME[name] = (
        max(dve_ops._SUB_OPCODE_FOR_NAME.values()) + 1)
    assert dve_ops._SUB_OPCODE_FOR_NAME[name] < 0x20
    for ver in ("v3", "v4"):
        try:
            op.compile(ver)
        except ValueError as e:
            m = re.search(r"\(%s: ([0-9a-f]+) " % ver, str(e))
            if not m:
                raise
            op.uops_sha[ver] = m.group(1)
            op.compile(ver)
    dve_ops.OPS.append(op)
    dve_ops.CUSTOM_DVE_SPECS[name] = op.spec
    return op


def _pin_combined_act_set(arch: str):
    """Make `natural_log_exp_and_others` the only ACT table set offering Exp
    and Ln, so Ln needs no extra load after the exp batches."""
    from concourse.hw_specs import get_activation_tables
    tabs = get_activation_tables(arch)
    assert AF.Exp in tabs["natural_log_exp_and_others"]
    assert AF.Ln in tabs["natural_log_exp_and_others"]
    for name, fns in tabs.items():
        if name != "natural_log_exp_and_others":
            fns.discard(AF.Exp)
            fns.discard(AF.Ln)


def build_bass(R_pad: int, nt2: int, nt2p: int):
    """R_pad: padded rows/core; nt2: window tiles; nt2p: dT slots (mult of 4)."""
    sqrelu_add = _register_sqrelu_add()
    CH = R_pad // 2           # psum free-dim chunk (<=512)
    assert CH <= 512
    nU = -(-R_pad // 128)     # 128-chunks of the local row range

    nc = bacc.Bacc("TRN2", target_bir_lowering=False, debug=False,
                   num_devices=NCORES)
    _pin_combined_act_set(nc.m.arch)

    # ---- kernel I/O (per-core shards prepared on host; j permuted) ----
    # at/bt2 are fp8e4m3: the -2ab cross term at fp8 perturbs the final
    # loss by ~1e-4 rel (validated offline); norm terms stay fp32 exact.
    at = nc.dram_tensor("at", [128, 4, N], FP8, kind="ExternalInput").ap()
    bt2 = nc.dram_tensor("bt2", [128, 4, R_pad], FP8, kind="ExternalInput").ap()
    aat = nc.dram_tensor("aat", [128, NT], F32, kind="ExternalInput").ap()
    bbv = nc.dram_tensor("bbv", [1, R_pad], F32, kind="ExternalInput").ap()
    oh3 = nc.dram_tensor("oh3", [128, NT * 3], BF16, kind="ExternalInput").ap()
    oh2 = nc.dram_tensor("oh2", [128, nt2 * 2], BF16, kind="ExternalInput").ap()
    sel3 = nc.dram_tensor("sel3", [3, R_pad], F32, kind="ExternalInput").ap()
    ohmy2 = nc.dram_tensor("ohmy2", [2, R_pad], F32, kind="ExternalInput").ap()
    ti = nc.dram_tensor("ti", [128, nU * nt2p], BF16, kind="ExternalInput").ap()
    # pm carries nU permutation blocks + one identity block (fill fold-in)
    pm = nc.dram_tensor("pm", [128, (nU + 1) * 128], BF16,
                        kind="ExternalInput").ap()
    fillm = nc.dram_tensor("fillm", [128, nt2p], BF16, kind="ExternalInput").ap()
    validm = nc.dram_tensor("validm", [1, R_pad], F32, kind="ExternalInput").ap()
    ddiag = nc.dram_tensor("ddiag", [1, R_pad], F32, kind="ExternalInput").ap()

    out_pos = nc.dram_tensor("out_pos", [2, 1], F32, kind="ExternalOutput").ap()
    out_diag = nc.dram_tensor("out_diag", [1, 1], F32, kind="ExternalOutput").ap()
    out_ns = nc.dram_tensor("out_ns", [1, R_pad], F32, kind="ExternalOutput").ap()

    with tile.TileContext(nc) as tc, ExitStack() as ctx:
        sb = ctx.enter_context(tc.tile_pool(name="sb", bufs=1))
        lp = ctx.enter_context(tc.tile_pool(name="lp", bufs=4))      # Dsq chunks f32
        dp = ctx.enter_context(tc.tile_pool(name="dp", bufs=6))      # non-window D f32
        ep = ctx.enter_context(tc.tile_pool(name="ep", bufs=4))      # Dexpm bf16
        work = ctx.enter_context(tc.tile_pool(name="work", bufs=2))
        small = ctx.enter_context(tc.tile_pool(name="small", bufs=2))
        tail = ctx.enter_context(tc.tile_pool(name="tail", bufs=1))

        # tiny dummy ACT op: forces the first (sqrt) table load during DMA wait
        dummy = sb.tile([1, 8], F32)
        nc.vector.memset(dummy, 1.0)
        last_sc = nc.scalar.activation(out=dummy, in_=dummy, func=AF.Sqrt)

        def chain_sc(inst):
            # explicit scalar-queue order: keeps sqrt/exp in table batches
            nonlocal last_sc
            add_dep_helper(inst.ins, last_sc.ins, False, "scalar batch order")
            last_sc = inst

        # ---- resident SBUF tensors / DMA issue plan ----
        # scalar queue: bb (tiny, feeds the on-device broadcast), then bt2
        bb_sb = sb.tile([1, R_pad], F32)
        nc.scalar.dma_start(out=bb_sb, in_=bbv)
        bt_sb = sb.tile([128, 4, R_pad], FP8)
        nc.scalar.dma_start(out=bt_sb, in_=bt2)
        # gpsimd queue (own DMA engine): aat first - the first stt needs it
        aat_sb = sb.tile([128, NT], F32)
        nc.gpsimd.dma_start(out=aat_sb, in_=aat)
        # sync queue: at fully resident (fp8 = 16KB/partition), split so
        # the first slots land fast and the tail keeps ahead of the GEMM
        at_sb = sb.tile([128, 4, N], FP8)
        for lo, hi in ((0, 256), (256, 1280), (1280, 2560), (2560, N)):
            nc.sync.dma_start(out=at_sb[:, :, lo:hi], in_=at[:, :, lo:hi])

        # remaining residents ride the tensor/vector queues mid-GEMM
        oh3_sb = sb.tile([128, NT * 3], BF16)
        oh2_sb = sb.tile([128, nt2 * 2], BF16)
        sel3_sb = sb.tile([3, R_pad], F32)
        ohmy2_sb = sb.tile([2, R_pad], F32)
        ti_sb = sb.tile([128, nU * nt2p], BF16)
        pm_sb = sb.tile([128, (nU + 1) * 128], BF16)
        fill_sb = sb.tile([128, nt2p], BF16)
        valid_sb = sb.tile([1, R_pad], F32)
        ddiag_sb = sb.tile([1, R_pad], F32)

        dT = sb.tile([128, nt2p, R_pad], F32)      # window D
        ones128c = sb.tile([1, 128], BF16)
        nc.vector.memset(ones128c, 1.0)
        ones128f = sb.tile([1, 128], F32)
        nc.vector.memset(ones128f, 1.0)
        ones11 = sb.tile([1, 1], F32)
        nc.vector.memset(ones11, 1.0)
        w3 = sb.tile([3, 1], BF16)
        nc.vector.memset(w3, -1.0)
        nc.vector.memset(w3[0:1], 1.0)

        # bb broadcast to all partitions built on-device (fp32-exact; saves
        # a 278KB DMA from the critical startup window)
        bbbc_sb = sb.tile([128, R_pad], F32)
        with tc.tile_pool(name="bc_ps", bufs=1, space="PSUM") as bc_pool:
            bcps = bc_pool.tile([128, 2, 512], F32)
            for c_ in range(2):
                nc.tensor.matmul(out=bcps[:, c_, 0:CH], lhsT=ones128f,
                                 rhs=bb_sb[:, c_ * CH:(c_ + 1) * CH],
                                 start=True, stop=True)
            nc.vector.tensor_copy(
                out=bbbc_sb.rearrange("p (c f) -> p c f", c=2),
                in_=bcps[:, :, 0:CH])

        # ================= PHASE 1: GEMM -> +norms -> sqrt -> exp =========
        with tc.tile_pool(name="bl_ps", bufs=1, space="PSUM") as bl_pool:
            dsq_ctx = tc.tile_pool(name="dsq_ps", bufs=3, space="PSUM")
            dsq_pool = dsq_ctx.__enter__()

            bl_ps = bl_pool.tile([3, 2, 512], F32)   # negsum accumulator

            L4 = None
            pend_D = []    # (D tile, first slot, n slots) awaiting exp
            pend_E = []    # (E tile, first slot, n slots) awaiting bylabel
            nbl = 0        # bylabel slots emitted (0..NT)

            def emit_bylabel():
                nonlocal nbl
                E4, t0, ntiles = pend_E.pop(0)
                for r_ in range(ntiles):
                    t = t0 + r_
                    for c_ in range(2):
                        nc.tensor.matmul(
                            out=bl_ps[:, c_, 0:CH],
                            lhsT=oh3_sb[:, t * 3:(t + 1) * 3],
                            rhs=E4[:, r_, c_ * CH:(c_ + 1) * CH],
                            start=(nbl == 0), stop=(nbl == NT - 1))
                    nbl += 1

            def emit_exp(n=100):
                while pend_D and n > 0:
                    D4b, t0b, csz_ = pend_D.pop(0)
                    E4 = ep.tile([128, csz_, R_pad], BF16, tag="E4")
                    chain_sc(nc.scalar.activation(
                        out=E4, in_=D4b, func=AF.Exp,
                        scale=-1.0, bias=float(MARGIN)))
                    pend_E.append((E4, t0b, csz_))
                    n -= 1

            # sqrt chunk plan: tiny chunks first (earlier ACT start),
            # then 4-slot chunks; table block 1 = chunks 0..2 (slots 0-3)
            chunk_sizes = [1, 1, 2] + [4] * 7
            chunk_start = [0]
            for csz_ in chunk_sizes[:-1]:
                chunk_start.append(chunk_start[-1] + csz_)
            slot2chunk = {}
            for ci_, (cs_, csz_) in enumerate(zip(chunk_start, chunk_sizes)):
                for o_ in range(csz_):
                    slot2chunk[cs_ + o_] = (ci_, o_, csz_)
            NBLK1 = 4

            for jt in range(NT):
                dsq = dsq_pool.tile([128, 2, 512], F32, tag="dsq")
                for c_ in range(2):
                    for g_ in range(2):
                        nc.tensor.matmul(
                            out=dsq[:, c_, 0:CH],
                            lhsT=at_sb[:, 2 * g_:2 * g_ + 2,
                                       jt * 128:(jt + 1) * 128],
                            rhs=bt_sb[:, 2 * g_:2 * g_ + 2,
                                      c_ * CH:(c_ + 1) * CH],
                            start=(g_ == 0), stop=(g_ == 1), perf_mode=DR)

                # resident DMA issues ride the (otherwise idle) gpsimd queue
                if jt == 1:
                    nc.gpsimd.dma_start(out=oh3_sb, in_=oh3)
                    nc.gpsimd.dma_start(out=pm_sb, in_=pm)
                    nc.gpsimd.dma_start(out=oh2_sb, in_=oh2)
                elif jt == 3:
                    nc.gpsimd.dma_start(out=sel3_sb, in_=sel3)
                    nc.gpsimd.dma_start(out=ohmy2_sb, in_=ohmy2)
                    nc.gpsimd.dma_start(out=ti_sb, in_=ti)
                elif jt == 5:
                    nc.gpsimd.dma_start(out=fill_sb, in_=fillm)
                    nc.gpsimd.dma_start(out=valid_sb, in_=validm)
                    nc.gpsimd.dma_start(out=ddiag_sb, in_=ddiag)

                # DVE adds the norm terms: L4 = dsq + aa[j] + bb[i]
                ci_, off_, csz_ = slot2chunk[jt]
                if off_ == 0:
                    L4 = lp.tile([128, csz_, R_pad], F32, tag="L4")
                nc.vector.scalar_tensor_tensor(
                    out=L4[:, off_, :].rearrange("p (c f) -> p c f", c=2),
                    in0=dsq[:, :, 0:CH],
                    scalar=aat_sb[:, jt:jt + 1],
                    in1=bbbc_sb.rearrange("p (c f) -> p c f", c=2),
                    op0=ALU.add, op1=ALU.add)

                # interleave bylabel matmuls for block-1 exps mid-GEMM
                if jt >= 12 and pend_E:
                    emit_bylabel()

                if off_ == csz_ - 1:
                    cs_ = chunk_start[ci_]
                    if cs_ + csz_ <= nt2p:
                        D4 = dT[:, cs_:cs_ + csz_, :]
                    else:
                        D4 = dp.tile([128, csz_, R_pad], F32, tag="D4")
                    chain_sc(nc.scalar.activation(out=D4, in_=L4,
                                                  func=AF.Sqrt))
                    pend_D.append((D4, cs_, csz_))
                    if ci_ == NBLK1 - 1:
                        emit_exp()     # exp chunks 0..NBLK1-1

            # sqrt chunks NBLK1.. happened above; now their exps with
            # bylabel trailing each exp so only the last chunk's bylabel
            # gates ns. The final chunk's exp is split in half so its
            # bylabel tail is ~1 us shorter.
            while pend_D:
                if len(pend_D) == 1:
                    # split the final chunk in half so only its tail
                    # slots' bylabel matmuls gate ns at the very end
                    D4b, t0b, csz_ = pend_D.pop(0)
                    parts = ([csz_ // 2, csz_ - csz_ // 2]
                             if csz_ > 1 else [csz_])
                    off_ = 0
                    for h_, psz_ in enumerate(parts):
                        E2 = ep.tile([128, psz_, R_pad], BF16, tag="E4",
                                     name=f"E2h{h_}")
                        chain_sc(nc.scalar.activation(
                            out=E2, in_=D4b[:, off_:off_ + psz_, :],
                            func=AF.Exp, scale=-1.0, bias=float(MARGIN)))
                        pend_E.append((E2, t0b + off_, psz_))
                        off_ += psz_
                        while len(pend_E) > 1:
                            emit_bylabel()
                else:
                    emit_exp(1)
                while len(pend_E) > 1:
                    emit_bylabel()
            while pend_E:
                emit_bylabel()

            dsq_ctx.__exit__(None, None, None)   # free the 6 dsq banks

            with tc.tile_pool(name="ns_ps", bufs=1, space="PSUM") as ns_pool:
                # -- ns = total - own-class:  w3.T @ (bl * sel3) --
                prod_sb = tail.tile([3, 2, CH], BF16, tag="prod3")
                nc.vector.scalar_tensor_tensor(
                    out=prod_sb, in0=bl_ps[:, :, 0:CH], scalar=0.0,
                    in1=sel3_sb.rearrange("p (c f) -> p c f", c=2),
                    op0=ALU.bypass, op1=ALU.mult)
                ns_ps = ns_pool.tile([1, 2, 512], F32, name="ns_ps")
                for c_ in range(2):
                    nc.tensor.matmul(out=ns_ps[:, c_, 0:CH], lhsT=w3,
                                     rhs=prod_sb[:, c_, :],
                                     start=True, stop=True)
                ns_my = sb.tile([1, R_pad], F32)
                nc.vector.tensor_copy(
                    out=ns_my.rearrange("p (c f) -> p c f", c=2),
                    in_=ns_ps[:, :, 0:CH])

                # diag ln(2 ns_i): fills the ACT idle window while the
                # nsT/ns_bc machinery runs
                lnterm = tail.tile([1, R_pad], F32, tag="lnt")
                chain_sc(nc.scalar.activation(out=lnterm, in_=ns_my,
                                              func=AF.Ln, scale=2.0))

                ns_bf = sb.tile([1, R_pad], BF16)
                nc.vector.tensor_copy(out=ns_bf, in_=ns_my)

                # -- nsT: ns_j in [128, slot] layout via transpose+perm --
                nsL_ps = ns_pool.tile([128, nU], F32, name="nsL_ps")
                for u in range(nU):
                    lo = 128 * u
                    hi = min(R_pad, lo + 128)
                    nc.tensor.matmul(out=nsL_ps[0:hi - lo, u:u + 1],
                                     lhsT=ns_my[0:1, lo:hi], rhs=ones11,
                                     start=True, stop=True)
                # broadcast ns_my across partitions: [128, R_pad]
                nsbc_ps = ns_pool.tile([128, 2, 512], F32, name="nsbc_ps")
                for c_ in range(2):
                    nc.tensor.matmul(out=nsbc_ps[:, c_, 0:CH], lhsT=ones128c,
                                     rhs=ns_bf[:, c_ * CH:(c_ + 1) * CH],
                                     start=True, stop=True)

                # rhs_u reads nsL straight from PSUM as the per-partition
                # scalar (garbage partitions are masked by ti=0, and psum
                # holds only finite floats); the trailing identity-block
                # matmul folds in the 1.0 fill for alien partitions.
                nsT_ps = ns_pool.tile([128, nt2p], F32, name="nsT_ps")
                for u in range(nU):
                    rhs_u = small.tile([128, nt2p], BF16, tag="rhsu")
                    nc.vector.scalar_tensor_tensor(
                        out=rhs_u, in0=ti_sb[:, u * nt2p:(u + 1) * nt2p],
                        scalar=nsL_ps[:, u:u + 1],
                        in1=ti_sb[:, u * nt2p:(u + 1) * nt2p],
                        op0=ALU.mult, op1=ALU.bypass)
                    nc.tensor.matmul(out=nsT_ps,
                                     lhsT=pm_sb[:, u * 128:(u + 1) * 128],
                                     rhs=rhs_u,
                                     start=(u == 0), stop=False)
                nc.tensor.matmul(out=nsT_ps,
                                 lhsT=pm_sb[:, nU * 128:(nU + 1) * 128],
                                 rhs=fill_sb, start=False, stop=True)
                # ACT bias APs must live in SBUF: one tiny copy
                nsT_sb = sb.tile([128, nt2p], F32)
                nc.vector.tensor_copy(out=nsT_sb, in_=nsT_ps)
                nc.sync.dma_start(out=out_ns, in_=ns_my)

                # ========= PHASE 2: J = ln(ns_i+ns_j) + D; hinge^2 =======
                # hinge-by-class accumulates into the (drained) bl banks
                for t in range(nt2):
                    Lt = work.tile([128, R_pad], F32, tag="L")
                    chain_sc(nc.scalar.activation(
                        out=Lt.rearrange("p (c f) -> p c f", c=2),
                        in_=nsbc_ps[:, :, 0:CH], func=AF.Ln,
                        bias=nsT_sb[:, t:t + 1], scale=1.0))
                    h2 = work.tile([128, R_pad], BF16, tag="h2")
                    acc_d = small.tile([128, 1], F32, tag="accd")
                    nc.vector._custom_dve(
                        sqrelu_add, out=h2, in0=Lt, in1=dT[:, t, :],
                        s0=0.0, accum_out=acc_d)
                    for c_ in range(2):
                        nc.tensor.matmul(
                            out=bl_ps[0:2, c_, 0:CH],
                            lhsT=oh2_sb[:, t * 2:(t + 1) * 2],
                            rhs=h2[:, c_ * CH:(c_ + 1) * CH],
                            start=(t == 0), stop=(t == nt2 - 1))

                # diagonal correction relu(ln(2 ns_i) + D_ii)^2 (masked)
                dh2 = tail.tile([1, R_pad], F32, tag="dh2")
                dummy_acc = small.tile([1, 1], F32, tag="dumacc")
                nc.vector._custom_dve(sqrelu_add, out=dh2, in0=lnterm,
                                      in1=ddiag_sb, s0=0.0,
                                      accum_out=dummy_acc)
                diag_acc = tail.tile([1, 1], F32, tag="dacc")
                dh2m = tail.tile([1, R_pad], F32, tag="dh2m")
                nc.vector.scalar_tensor_tensor(
                    out=dh2m, in0=dh2, scalar=0.0, in1=valid_sb,
                    op0=ALU.bypass, op1=ALU.mult, accum_out=diag_acc)
                nc.sync.dma_start(out=out_diag, in_=diag_acc)

                # -- combine: mask by i-side class match, accumulate --
                prod2 = tail.tile([2, 2, CH], F32, tag="prod2")
                acc2 = small.tile([2, 1], F32, tag="acc2")
                nc.vector.scalar_tensor_tensor(
                    out=prod2, in0=bl_ps[0:2, :, 0:CH], scalar=0.0,
                    in1=ohmy2_sb.rearrange("p (c f) -> p c f", c=2),
                    op0=ALU.bypass, op1=ALU.mult, accum_out=acc2)
                nc.sync.dma_start(out=out_pos, in_=acc2)

    nc.compile()
    return nc


_CACHE: dict = {}


def _get_nc(R_pad: int, nt2: int, nt2p: int):
    key = ("nc", R_pad, nt2, nt2p)
    if key not in _CACHE:
        _CACHE[key] = build_bass(R_pad, nt2, nt2p)
    return _CACHE[key]


def prepare_inputs(a: np.ndarray, b: np.ndarray, labels: np.ndarray):
    """Host-side label sort, class pairing, per-core shard + mask prep.

    Returns (per-core input maps, (R_pad, nt2, nt2p), meta)."""
    a = np.asarray(a, np.float32)
    b = np.asarray(b, np.float32)
    labels = np.asarray(labels)

    order = np.argsort(labels, kind="stable")
    a_s = a[order]
    b_s = b[order]
    sl = labels[order]
    counts = np.bincount(sl.astype(np.int64), minlength=NCLS)
    startscum = np.concatenate([[0], np.cumsum(counts)])

    def pair_tiles(p, q):
        ta = set(range(int(startscum[p]) // 128,
                       -(-int(startscum[p + 1]) // 128)))
        tb = set(range(int(startscum[q]) // 128,
                       -(-int(startscum[q + 1]) // 128)))
        return len(ta | tb)

    def pairing_cost(pairs_):
        return (max(int(counts[p] + counts[q]) for p, q in pairs_),
                max(pair_tiles(p, q) for p, q in pairs_))

    # greedy largest-with-smallest, then 2-opt swaps minimizing
    # (max pair size, max window tiles) lexicographically
    co = np.argsort(counts)
    pairs = [(int(co[i]), int(co[NCLS - 1 - i])) for i in range(NCORES)]
    best = pairing_cost(pairs)
    improved = True
    while improved:
        improved = False
        for i in range(NCORES):
            for j in range(i + 1, NCORES):
                for swap in ((0, 0), (0, 1)):
                    cand = list(pairs)
                    a1, b1 = pairs[i]
                    a2, b2 = pairs[j]
                    if swap == (0, 0):
                        cand[i], cand[j] = (a2, b1), (a1, b2)
                    else:
                        cand[i], cand[j] = (b2, b1), (a2, a1)
                    c = pairing_cost(cand)
                    if c < best:
                        pairs, best, improved = cand, c, True
    R_pad = best[0]
    R_pad = -(-R_pad // 32) * 32
    nU = -(-R_pad // 128)

    cores = []
    nt2 = 0
    for p, q in pairs:
        grows = np.concatenate([
            np.arange(startscum[p], startscum[p + 1]),
            np.arange(startscum[q], startscum[q + 1])])
        wtiles = sorted(set((grows // 128).tolist()))
        nt2 = max(nt2, len(wtiles))
        cores.append((p, q, grows, wtiles))
    nt2p = -(-nt2 // 4) * 4

    at_full = np.ascontiguousarray(a_s.T)                  # [F, N] sorted
    aa = np.sum(a_s * a_s, axis=1, dtype=np.float32)
    bb_s = np.sum(b_s * b_s, axis=1, dtype=np.float32)

    in_maps = []
    meta = []
    for c in range(NCORES):
        p, q, grows, wtiles = cores[c]
        Rc = len(grows)
        rest = [t for t in range(NT) if t not in wtiles]
        sigma = np.array(list(wtiles) + rest)
        slot_of = {t: s_ for s_, t in enumerate(sigma)}

        cols = (sigma[:, None] * 128 + np.arange(128)[None, :]).reshape(-1)
        # [128 kpart, 4 ksub, N] fp8 for DoubleRow lhsT slices
        at_c = np.ascontiguousarray(
            at_full[:, cols].reshape(4, 128, N).transpose(1, 0, 2)
        ).astype(NPFP8)
        aat_c = np.ascontiguousarray(aa[cols].reshape(NT, 128).T)  # [128, NT]

        glbl = sl[cols].reshape(NT, 128)                   # labels per slot
        oh3_c = np.zeros((NT, 128, 3), np.float32)
        oh3_c[:, :, 0] = 1.0
        oh3_c[:, :, 1] = glbl == p
        oh3_c[:, :, 2] = glbl == q
        oh3_c = np.ascontiguousarray(
            oh3_c.transpose(1, 0, 2).reshape(128, NT * 3)).astype(NPBF16)
        oh2_c = np.zeros((nt2, 128, 2), np.float32)
        oh2_c[:, :, 0] = glbl[:nt2] == p
        oh2_c[:, :, 1] = glbl[:nt2] == q
        oh2_c = np.ascontiguousarray(
            oh2_c.transpose(1, 0, 2).reshape(128, nt2 * 2)).astype(NPBF16)

        b_loc = np.zeros((R_pad, F), np.float32)
        b_loc[:Rc] = b_s[grows]
        a_my = np.zeros((R_pad, F), np.float32)
        a_my[:Rc] = a_s[grows]
        bb_loc = np.zeros(R_pad, np.float32)
        bb_loc[:Rc] = bb_s[grows]
        bt2_c = np.ascontiguousarray(
            (-2.0 * b_loc).T.reshape(4, 128, R_pad).transpose(1, 0, 2)
        ).astype(NPFP8)
        bbv_c = bb_loc.reshape(1, R_pad).copy()

        lbl_loc = np.full(R_pad, -1, np.int64)
        lbl_loc[:Rc] = sl[grows]
        selP = (lbl_loc == p).astype(np.float32)
        selQ = (lbl_loc == q).astype(np.float32)
        sel3_c = np.ascontiguousarray(
            np.stack([np.ones(R_pad, np.float32), selP, selQ], 0))
        ohmy2_c = np.ascontiguousarray(np.stack([selP, selQ], 0))

        ti_c = np.zeros((nU, 128, nt2p), np.float32)
        pm_c = np.zeros((nU + 1, 128, 128), np.float32)
        pm_c[nU] = np.eye(128, dtype=np.float32)
        used = np.zeros((128, nt2p), bool)
        for r in range(Rc):
            gr = grows[r]
            u, cc = r // 128, r % 128
            t_ = slot_of[gr // 128]
            ti_c[u, cc, t_] = 1.0
            pm_c[u, cc, gr % 128] = 1.0
            used[gr % 128, t_] = True
        ti_c = np.ascontiguousarray(
            ti_c.transpose(1, 0, 2).reshape(128, nU * nt2p)).astype(NPBF16)
        pm_c = np.ascontiguousarray(
            pm_c.transpose(1, 0, 2).reshape(128, (nU + 1) * 128)).astype(NPBF16)
        fill_c = np.where(used, 0.0, 1.0).astype(NPBF16)

        valid_c = (np.arange(R_pad) < Rc).astype(np.float32).reshape(1, R_pad)
        dd = np.sum(np.square(b_loc - a_my), axis=1, dtype=np.float32)
        ddiag_c = np.sqrt(np.maximum(dd, 0.0)).reshape(1, R_pad)

        in_maps.append({
            "at": at_c, "bt2": bt2_c, "aat": aat_c, "bbv": bbv_c,
            "oh3": oh3_c, "oh2": oh2_c, "sel3": sel3_c, "ohmy2": ohmy2_c,
            "ti": ti_c, "pm": pm_c, "fillm": np.ascontiguousarray(fill_c),
            "validm": valid_c, "ddiag": ddiag_c,
        })
        meta.append({"grows": grows, "Rc": Rc})
    return in_maps, (R_pad, nt2, nt2p), {"order": order, "cores": meta}


def run(a, b, labels, trace=False, trace_kwargs=None):
    """Run on 8 NeuronCores; returns (loss, BassKernelResults, meta)."""
    in_maps, dims, meta = prepare_inputs(a, b, labels)
    nc = _get_nc(*dims)
    kw = {}
    if trace:
        kw = dict(trace=True, **(trace_kwargs or {}))
    res = run_bass_kernel_spmd(nc, in_maps, core_ids=list(range(NCORES)), **kw)

    counts = np.bincount(np.asarray(labels).astype(np.int64), minlength=NCLS)
    num_pos = float((counts.astype(np.float64) ** 2).sum() - N)

    total = 0.0
    for c in range(NCORES):
        r = res.results[c]
        total += (float(r["out_pos"][0, 0]) + float(r["out_pos"][1, 0])
                  - float(r["out_diag"][0, 0]))
    loss = total / (2.0 * num_pos)
    return np.asarray(np.float32(loss)), res, meta


def kernel(a, b, labels):
    loss, _, _ = run(a, b, labels)
    return loss


# revision 54
# speedup vs baseline: 1.1190x; 1.1190x over previous
"""Trainium2 Bass kernel for nn_MetricLoss (lifted-structure-style metric loss).

Reference computation (N=4096 rows, F=512 features, 16 label classes):
    Dsq = ||b_i||^2 + ||a_j||^2 - 2 b@a.T ;  D = sqrt(max(Dsq,0))   [N,N]
    Dexpm = exp(1 - D)
    row_negsum[i] = sum_{j: lbl_j != lbl_i} Dexpm[i,j]
    J = log(row_negsum[i] + row_negsum[j]) + D
    loss = sum_{i!=j, lbl_i==lbl_j} relu(J)^2 / (2 * num_pos)

Design — fully decoupled cores (NO collectives), measured 73.8us max-core
vs the v1 collective baseline's 123-210us (v1's per-core time included
multi-core launch-skew waits at its AllGather; here cores never wait on
each other, so per-core exec time is just own work and run-to-run
variance collapses to <2us):
  * Rows are sorted by label on the host; label classes are PAIRED
    (2-opt search minimizing (max pair rows, max window tiles)) and each
    core owns ALL rows of its 2 classes, padded with zero-rows to a
    common R_pad (544 here). Every positive pair (i, j) then has both
    ns_i and ns_j computed locally - the AllGather of row_negsum is gone.
  * Per-core column permutation sigma puts the core's window j-tiles
    (tiles overlapping its 2 classes) at slots 0..nt2-1, so the phase-2
    loop structure is core-independent (pure SPMD); all class masks are
    input data, not program structure.
  * The -2*b@a.T GEMM runs in fp8e4m3 with DoubleRow perf mode (0.5
    cycles/row, 2 matmuls per slot-chunk contracting 256 k each); the
    fp8 cross term perturbs the final loss by ~1.2e-4 rel (validated
    offline against the fp64 reference). Norm terms stay fp32-exact:
    aa[j] rides the DVE add as a per-partition scalar, bb[i] as a
    broadcast tile built on-device by an fp32 ones-matmul (keeps 278KB
    off the startup DMA window). `at` is fully SBUF-resident (fp8 =
    16KB/partition), DMA'd in 4 column splits so slot 0 lands early.
  * negsum via 3-column one-hot matmuls per j-tile (ones/classP/classQ);
    ns = total - own-class, combined with a [3,1] +-1 bf16 matmul.
  * ns_j in partition layout (nsT[128, slot]) is built with 5 tiny
    transpose matmuls + per-128-chunk (TI mask x PM permutation-matmul)
    accumulation; a trailing identity-block matmul folds in the 1.0
    fill for alien partitions. All SPMD-safe despite per-core offsets.
  * D_ii (diagonal) is host-precomputed (same O(N F) class as aa/bb).
  * ACT (the critical engine, ~44us busy: sqrt+exp over 2.2M elems plus
    phase-2 Ln) runs in 4 table blocks - sqrt chunks {1,1,2,4}, exp
    those, sqrt the rest (stt-cadence-gated), exp the rest - with the
    last exp split in half so its negsum matmuls gate ns ~1us less.
    Tiny first chunks start the sqrt stream ~1.5us earlier. ln(2 ns)
    for the diagonal fills the ACT idle gap while the nsT/ns-broadcast
    machinery runs; phase-2 Ln reads the ns broadcast straight from
    PSUM (only the bias requires SBUF). Exp/Ln share one table set
    (natural_log_exp_and_others pinned), so 3 loads total after the
    preamble sqrt load.
  * Phase-2 hinge-by-class matmuls accumulate into the drained negsum
    PSUM banks, keeping total PSUM use at exactly 8 banks with no
    serialization on pool space.
  * GEMM free dim is chunked 2x272 (R_pad=544) so matmul outputs stay
    within PSUM banks.
"""

import re
import operator
import numpy as np
import ml_dtypes
from contextlib import ExitStack

import concourse.bass as bass
import concourse.tile as tile
from concourse import bacc, mybir
from concourse import dve_ops
from concourse.dve_spec import Spec, Src0, Src1, C0, relu, sq
from concourse.bass_utils import run_bass_kernel_spmd
from concourse.tile_rust import add_dep_helper

F32 = mybir.dt.float32
BF16 = mybir.dt.bfloat16
FP8 = mybir.dt.float8e4
NPBF16 = ml_dtypes.bfloat16
NPFP8 = mybir.dt.np(mybir.dt.float8e4)
AF = mybir.ActivationFunctionType
ALU = mybir.AluOpType
DR = mybir.MatmulPerfMode.DoubleRow

N = 4096          # rows (a and b)
F = 512           # features
NCORES = 8
NT = N // 128     # j-tiles of 128 partitions = 32
NCLS = 16         # label classes
MARGIN = 1.0


def _register_sqrelu_add():
    """Custom fused DVE op: out = relu(in0 + in1)^2, accum_out = c0 + sum(out)."""
    name = "SQRELU_ADD_ANT"
    for op in dve_ops.OPS:
        if op.name == name:
            return op
    op = dve_ops.DveOp(
        name,
        Spec(body=sq(relu(Src0 + Src1)), accum=operator.add, accum_init=C0),
        subdim=False,
        uops_sha={},
    )
    dve_ops._SUB_OPCODE_FOR_NAME[name] = (
        max(dve_ops._SUB_OPCODE_FOR_NAME.values()) + 1)
    assert dve_ops._SUB_OPCODE_FOR_NAME[name] < 0x20
    for ver in ("v3", "v4"):
        try:
            op.compile(ver)
        except ValueError as e:
            m = re.search(r"\(%s: ([0-9a-f]+) " % ver, str(e))
            if not m:
                raise
            op.uops_sha[ver] = m.group(1)
            op.compile(ver)
    dve_ops.OPS.append(op)
    dve_ops.CUSTOM_DVE_SPECS[name] = op.spec
    return op


def _pin_combined_act_set(arch: str):
    """Make `natural_log_exp_and_others` the only ACT table set offering Exp
    and Ln, so Ln needs no extra load after the exp batches."""
    from concourse.hw_specs import get_activation_tables
    tabs = get_activation_tables(arch)
    assert AF.Exp in tabs["natural_log_exp_and_others"]
    assert AF.Ln in tabs["natural_log_exp_and_others"]
    for name, fns in tabs.items():
        if name != "natural_log_exp_and_others":
            fns.discard(AF.Exp)
            fns.discard(AF.Ln)


def build_bass(R_pad: int, nt2: int, nt2p: int):
    """R_pad: padded rows/core; nt2: window tiles; nt2p: dT slots (mult of 4)."""
    sqrelu_add = _register_sqrelu_add()
    CH = R_pad // 2           # psum free-dim chunk (<=512)
    assert CH <= 512
    nU = -(-R_pad // 128)     # 128-chunks of the local row range

    nc = bacc.Bacc("TRN2", target_bir_lowering=False, debug=False,
                   num_devices=NCORES)
    _pin_combined_act_set(nc.m.arch)

    # ---- kernel I/O (per-core shards prepared on host; j permuted) ----
    # at/bt2 are fp8e4m3: the -2ab cross term at fp8 perturbs the final
    # loss by ~1e-4 rel (validated offline); norm terms stay fp32 exact.
    at = nc.dram_tensor("at", [128, 4, N], FP8, kind="ExternalInput").ap()
    bt2 = nc.dram_tensor("bt2", [128, 4, R_pad], FP8, kind="ExternalInput").ap()
    aat = nc.dram_tensor("aat", [128, NT], F32, kind="ExternalInput").ap()
    bbv = nc.dram_tensor("bbv", [1, R_pad], F32, kind="ExternalInput").ap()
    oh3 = nc.dram_tensor("oh3", [128, NT * 3], BF16, kind="ExternalInput").ap()
    oh2 = nc.dram_tensor("oh2", [128, nt2 * 2], BF16, kind="ExternalInput").ap()
    sel3 = nc.dram_tensor("sel3", [3, R_pad], F32, kind="ExternalInput").ap()
    ohmy2 = nc.dram_tensor("ohmy2", [2, R_pad], F32, kind="ExternalInput").ap()
    ti = nc.dram_tensor("ti", [128, nU * nt2p], BF16, kind="ExternalInput").ap()
    # pm carries nU permutation blocks + one identity block (fill fold-in)
    pm = nc.dram_tensor("pm", [128, (nU + 1) * 128], BF16,
                        kind="ExternalInput").ap()
    fillm = nc.dram_tensor("fillm", [128, nt2p], BF16, kind="ExternalInput").ap()
    validm = nc.dram_tensor("validm", [1, R_pad], F32, kind="ExternalInput").ap()
    ddiag = nc.dram_tensor("ddiag", [1, R_pad], F32, kind="ExternalInput").ap()

    out_pos = nc.dram_tensor("out_pos", [2, 1], F32, kind="ExternalOutput").ap()
    out_diag = nc.dram_tensor("out_diag", [1, 1], F32, kind="ExternalOutput").ap()
    out_ns = nc.dram_tensor("out_ns", [1, R_pad], F32, kind="ExternalOutput").ap()

    with tile.TileContext(nc) as tc, ExitStack() as ctx:
        sb = ctx.enter_context(tc.tile_pool(name="sb", bufs=1))
        lp = ctx.enter_context(tc.tile_pool(name="lp", bufs=4))      # Dsq chunks f32
        dp = ctx.enter_context(tc.tile_pool(name="dp", bufs=6))      # non-window D f32
        ep = ctx.enter_context(tc.tile_pool(name="ep", bufs=4))      # Dexpm bf16
        work = ctx.enter_context(tc.tile_pool(name="work", bufs=2))
        small = ctx.enter_context(tc.tile_pool(name="small", bufs=2))
        tail = ctx.enter_context(tc.tile_pool(name="tail", bufs=1))

        # tiny dummy ACT op: forces the first (sqrt) table load during DMA wait
        dummy = sb.tile([1, 8], F32)
        nc.vector.memset(dummy, 1.0)
        last_sc = nc.scalar.activation(out=dummy, in_=dummy, func=AF.Sqrt)

        def chain_sc(inst):
            # explicit scalar-queue order: keeps sqrt/exp in table batches
            nonlocal last_sc
            add_dep_helper(inst.ins, last_sc.ins, False, "scalar batch order")
            last_sc = inst

        # ---- resident SBUF tensors / DMA issue plan ----
        # scalar queue: bt2 first (gates the first GEMM matmul), then the
        # tiny bb vector that feeds the on-device broadcast
        bt_sb = sb.tile([128, 4, R_pad], FP8)
        nc.scalar.dma_start(out=bt_sb, in_=bt2)
        bb_sb = sb.tile([1, R_pad], F32)
        nc.scalar.dma_start(out=bb_sb, in_=bbv)
        # gpsimd queue (own DMA engine): aat first - the first stt needs it
        aat_sb = sb.tile([128, NT], F32)
        nc.gpsimd.dma_start(out=aat_sb, in_=aat)
        # sync queue: at fully resident (fp8 = 16KB/partition), split so
        # the first slots land fast and the tail keeps ahead of the GEMM
        at_sb = sb.tile([128, 4, N], FP8)
        for lo, hi in ((0, 256), (256, 1280), (1280, 2560), (2560, N)):
            nc.sync.dma_start(out=at_sb[:, :, lo:hi], in_=at[:, :, lo:hi])

        # remaining residents ride the tensor/vector queues mid-GEMM
        oh3_sb = sb.tile([128, NT * 3], BF16)
        oh2_sb = sb.tile([128, nt2 * 2], BF16)
        sel3_sb = sb.tile([3, R_pad], F32)
        ohmy2_sb = sb.tile([2, R_pad], F32)
        ti_sb = sb.tile([128, nU * nt2p], BF16)
        pm_sb = sb.tile([128, (nU + 1) * 128], BF16)
        fill_sb = sb.tile([128, nt2p], BF16)
        valid_sb = sb.tile([1, R_pad], F32)
        ddiag_sb = sb.tile([1, R_pad], F32)

        dT = sb.tile([128, nt2p, R_pad], F32)      # window D
        ones128c = sb.tile([1, 128], BF16)
        nc.vector.memset(ones128c, 1.0)
        ones128f = sb.tile([1, 128], F32)
        nc.vector.memset(ones128f, 1.0)
        ones11 = sb.tile([1, 1], F32)
        nc.vector.memset(ones11, 1.0)
        w3 = sb.tile([3, 1], BF16)
        nc.vector.memset(w3, -1.0)
        nc.vector.memset(w3[0:1], 1.0)

        # bb broadcast to all partitions built on-device (fp32-exact; saves
        # a 278KB DMA from the critical startup window)
        bbbc_sb = sb.tile([128, R_pad], F32)
        with tc.tile_pool(name="bc_ps", bufs=1, space="PSUM") as bc_pool:
            bcps = bc_pool.tile([128, 2, 512], F32)
            for c_ in range(2):
                nc.tensor.matmul(out=bcps[:, c_, 0:CH], lhsT=ones128f,
                                 rhs=bb_sb[:, c_ * CH:(c_ + 1) * CH],
                                 start=True, stop=True)
            nc.vector.tensor_copy(
                out=bbbc_sb.rearrange("p (c f) -> p c f", c=2),
                in_=bcps[:, :, 0:CH])

        # ================= PHASE 1: GEMM -> +norms -> sqrt -> exp =========
        with tc.tile_pool(name="bl_ps", bufs=1, space="PSUM") as bl_pool:
            dsq_ctx = tc.tile_pool(name="dsq_ps", bufs=3, space="PSUM")
            dsq_pool = dsq_ctx.__enter__()

            bl_ps = bl_pool.tile([3, 2, 512], F32)   # negsum accumulator

            L4 = None
            pend_D = []    # (D tile, first slot, n slots) awaiting exp
            pend_E = []    # (E tile, first slot, n slots) awaiting bylabel
            nbl = 0        # bylabel slots emitted (0..NT)

            def emit_bylabel():
                nonlocal nbl
                E4, t0, ntiles = pend_E.pop(0)
                for r_ in range(ntiles):
                    t = t0 + r_
                    for c_ in range(2):
                        nc.tensor.matmul(
                            out=bl_ps[:, c_, 0:CH],
                            lhsT=oh3_sb[:, t * 3:(t + 1) * 3],
                            rhs=E4[:, r_, c_ * CH:(c_ + 1) * CH],
                            start=(nbl == 0), stop=(nbl == NT - 1))
                    nbl += 1

            def emit_exp(n=100):
                while pend_D and n > 0:
                    D4b, t0b, csz_ = pend_D.pop(0)
                    E4 = ep.tile([128, csz_, R_pad], BF16, tag="E4")
                    chain_sc(nc.scalar.activation(
                        out=E4, in_=D4b, func=AF.Exp,
                        scale=-1.0, bias=float(MARGIN)))
                    pend_E.append((E4, t0b, csz_))
                    n -= 1

            # sqrt chunk plan: tiny chunks first (earlier ACT start),
            # then 4-slot chunks; table block 1 = chunks 0..2 (slots 0-3)
            chunk_sizes = [1, 1, 2] + [4] * 7
            chunk_start = [0]
            for csz_ in chunk_sizes[:-1]:
                chunk_start.append(chunk_start[-1] + csz_)
            slot2chunk = {}
            for ci_, (cs_, csz_) in enumerate(zip(chunk_start, chunk_sizes)):
                for o_ in range(csz_):
                    slot2chunk[cs_ + o_] = (ci_, o_, csz_)
            NBLK1 = 4

            for jt in range(NT):
                dsq = dsq_pool.tile([128, 2, 512], F32, tag="dsq")
                for c_ in range(2):
                    for g_ in range(2):
                        nc.tensor.matmul(
                            out=dsq[:, c_, 0:CH],
                            lhsT=at_sb[:, 2 * g_:2 * g_ + 2,
                                       jt * 128:(jt + 1) * 128],
                            rhs=bt_sb[:, 2 * g_:2 * g_ + 2,
                                      c_ * CH:(c_ + 1) * CH],
                            start=(g_ == 0), stop=(g_ == 1), perf_mode=DR)

                # resident DMA issues ride the (otherwise idle) gpsimd queue
                if jt == 1:
                    nc.gpsimd.dma_start(out=oh3_sb, in_=oh3)
                    nc.gpsimd.dma_start(out=pm_sb, in_=pm)
                    nc.gpsimd.dma_start(out=oh2_sb, in_=oh2)
                elif jt == 3:
                    nc.gpsimd.dma_start(out=sel3_sb, in_=sel3)
                    nc.gpsimd.dma_start(out=ohmy2_sb, in_=ohmy2)
                    nc.gpsimd.dma_start(out=ti_sb, in_=ti)
                elif jt == 5:
                    nc.gpsimd.dma_start(out=fill_sb, in_=fillm)
                    nc.gpsimd.dma_start(out=valid_sb, in_=validm)
                    nc.gpsimd.dma_start(out=ddiag_sb, in_=ddiag)

                # DVE adds the norm terms: L4 = dsq + aa[j] + bb[i]
                ci_, off_, csz_ = slot2chunk[jt]
                if off_ == 0:
                    L4 = lp.tile([128, csz_, R_pad], F32, tag="L4")
                nc.vector.scalar_tensor_tensor(
                    out=L4[:, off_, :].rearrange("p (c f) -> p c f", c=2),
                    in0=dsq[:, :, 0:CH],
                    scalar=aat_sb[:, jt:jt + 1],
                    in1=bbbc_sb.rearrange("p (c f) -> p c f", c=2),
                    op0=ALU.add, op1=ALU.add)

                # interleave bylabel matmuls for block-1 exps mid-GEMM
                if jt >= 12 and pend_E:
                    emit_bylabel()

                if off_ == csz_ - 1:
                    cs_ = chunk_start[ci_]
                    if cs_ + csz_ <= nt2p:
                        D4 = dT[:, cs_:cs_ + csz_, :]
                    else:
                        D4 = dp.tile([128, csz_, R_pad], F32, tag="D4")
                    chain_sc(nc.scalar.activation(out=D4, in_=L4,
                                                  func=AF.Sqrt))
                    pend_D.append((D4, cs_, csz_))
                    if ci_ == NBLK1 - 1:
                        emit_exp()     # exp chunks 0..NBLK1-1

            # sqrt chunks NBLK1.. happened above; now their exps with
            # bylabel trailing each exp so only the last chunk's bylabel
            # gates ns. The final chunk's exp is split in half so its
            # bylabel tail is ~1 us shorter.
            while pend_D:
                if len(pend_D) == 1:
                    # split the final chunk in half so only its tail
                    # slots' bylabel matmuls gate ns at the very end
                    D4b, t0b, csz_ = pend_D.pop(0)
                    parts = ([csz_ // 2, csz_ - csz_ // 2]
                             if csz_ > 1 else [csz_])
                    off_ = 0
                    for h_, psz_ in enumerate(parts):
                        E2 = ep.tile([128, psz_, R_pad], BF16, tag="E4",
                                     name=f"E2h{h_}")
                        chain_sc(nc.scalar.activation(
                            out=E2, in_=D4b[:, off_:off_ + psz_, :],
                            func=AF.Exp, scale=-1.0, bias=float(MARGIN)))
                        pend_E.append((E2, t0b + off_, psz_))
                        off_ += psz_
                        while len(pend_E) > 1:
                            emit_bylabel()
                else:
                    emit_exp(1)
                while len(pend_E) > 1:
                    emit_bylabel()
            while pend_E:
                emit_bylabel()

            dsq_ctx.__exit__(None, None, None)   # free the 6 dsq banks

            with tc.tile_pool(name="ns_ps", bufs=1, space="PSUM") as ns_pool:
                # -- ns = total - own-class:  w3.T @ (bl * sel3) --
                prod_sb = tail.tile([3, 2, CH], BF16, tag="prod3")
                nc.vector.scalar_tensor_tensor(
                    out=prod_sb, in0=bl_ps[:, :, 0:CH], scalar=0.0,
                    in1=sel3_sb.rearrange("p (c f) -> p c f", c=2),
                    op0=ALU.bypass, op1=ALU.mult)
                ns_ps = ns_pool.tile([1, 2, 512], F32, name="ns_ps")
                for c_ in range(2):
                    nc.tensor.matmul(out=ns_ps[:, c_, 0:CH], lhsT=w3,
                                     rhs=prod_sb[:, c_, :],
                                     start=True, stop=True)
                ns_my = sb.tile([1, R_pad], F32)
                nc.vector.tensor_copy(
                    out=ns_my.rearrange("p (c f) -> p c f", c=2),
                    in_=ns_ps[:, :, 0:CH])

                # diag ln(2 ns_i): fills the ACT idle window while the
                # nsT/ns_bc machinery runs
                lnterm = tail.tile([1, R_pad], F32, tag="lnt")
                chain_sc(nc.scalar.activation(out=lnterm, in_=ns_my,
                                              func=AF.Ln, scale=2.0))

                ns_bf = sb.tile([1, R_pad], BF16)
                nc.vector.tensor_copy(out=ns_bf, in_=ns_my)

                # -- nsT: ns_j in [128, slot] layout via transpose+perm --
                nsL_ps = ns_pool.tile([128, nU], F32, name="nsL_ps")
                for u in range(nU):
                    lo = 128 * u
                    hi = min(R_pad, lo + 128)
                    nc.tensor.matmul(out=nsL_ps[0:hi - lo, u:u + 1],
                                     lhsT=ns_my[0:1, lo:hi], rhs=ones11,
                                     start=True, stop=True)
                # broadcast ns_my across partitions: [128, R_pad]
                nsbc_ps = ns_pool.tile([128, 2, 512], F32, name="nsbc_ps")
                for c_ in range(2):
                    nc.tensor.matmul(out=nsbc_ps[:, c_, 0:CH], lhsT=ones128c,
                                     rhs=ns_bf[:, c_ * CH:(c_ + 1) * CH],
                                     start=True, stop=True)

                # rhs_u reads nsL straight from PSUM as the per-partition
                # scalar (garbage partitions are masked by ti=0, and psum
                # holds only finite floats); the trailing identity-block
                # matmul folds in the 1.0 fill for alien partitions.
                nsT_ps = ns_pool.tile([128, nt2p], F32, name="nsT_ps")
                for u in range(nU):
                    rhs_u = small.tile([128, nt2p], BF16, tag="rhsu")
                    nc.vector.scalar_tensor_tensor(
                        out=rhs_u, in0=ti_sb[:, u * nt2p:(u + 1) * nt2p],
                        scalar=nsL_ps[:, u:u + 1],
                        in1=ti_sb[:, u * nt2p:(u + 1) * nt2p],
                        op0=ALU.mult, op1=ALU.bypass)
                    nc.tensor.matmul(out=nsT_ps,
                                     lhsT=pm_sb[:, u * 128:(u + 1) * 128],
                                     rhs=rhs_u,
                                     start=(u == 0), stop=False)
                nc.tensor.matmul(out=nsT_ps,
                                 lhsT=pm_sb[:, nU * 128:(nU + 1) * 128],
                                 rhs=fill_sb, start=False, stop=True)
                # ACT bias APs must live in SBUF: one tiny copy
                nsT_sb = sb.tile([128, nt2p], F32)
                nc.vector.tensor_copy(out=nsT_sb, in_=nsT_ps)
                nc.sync.dma_start(out=out_ns, in_=ns_my)

                # ========= PHASE 2: J = ln(ns_i+ns_j) + D; hinge^2 =======
                # hinge-by-class accumulates into the (drained) bl banks
                for t in range(nt2):
                    Lt = work.tile([128, R_pad], F32, tag="L")
                    chain_sc(nc.scalar.activation(
                        out=Lt.rearrange("p (c f) -> p c f", c=2),
                        in_=nsbc_ps[:, :, 0:CH], func=AF.Ln,
                        bias=nsT_sb[:, t:t + 1], scale=1.0))
                    h2 = work.tile([128, R_pad], BF16, tag="h2")
                    acc_d = small.tile([128, 1], F32, tag="accd")
                    nc.vector._custom_dve(
                        sqrelu_add, out=h2, in0=Lt, in1=dT[:, t, :],
                        s0=0.0, accum_out=acc_d)
                    for c_ in range(2):
                        nc.tensor.matmul(
                            out=bl_ps[0:2, c_, 0:CH],
                            lhsT=oh2_sb[:, t * 2:(t + 1) * 2],
                            rhs=h2[:, c_ * CH:(c_ + 1) * CH],
                            start=(t == 0), stop=(t == nt2 - 1))

                # diagonal correction relu(ln(2 ns_i) + D_ii)^2 (masked)
                dh2 = tail.tile([1, R_pad], F32, tag="dh2")
                dummy_acc = small.tile([1, 1], F32, tag="dumacc")
                nc.vector._custom_dve(sqrelu_add, out=dh2, in0=lnterm,
                                      in1=ddiag_sb, s0=0.0,
                                      accum_out=dummy_acc)
                diag_acc = tail.tile([1, 1], F32, tag="dacc")
                dh2m = tail.tile([1, R_pad], F32, tag="dh2m")
                nc.vector.scalar_tensor_tensor(
                    out=dh2m, in0=dh2, scalar=0.0, in1=valid_sb,
                    op0=ALU.bypass, op1=ALU.mult, accum_out=diag_acc)
                nc.sync.dma_start(out=out_diag, in_=diag_acc)

                # -- combine: mask by i-side class match, accumulate --
                prod2 = tail.tile([2, 2, CH], F32, tag="prod2")
                acc2 = small.tile([2, 1], F32, tag="acc2")
                nc.vector.scalar_tensor_tensor(
                    out=prod2, in0=bl_ps[0:2, :, 0:CH], scalar=0.0,
                    in1=ohmy2_sb.rearrange("p (c f) -> p c f", c=2),
                    op0=ALU.bypass, op1=ALU.mult, accum_out=acc2)
                nc.sync.dma_start(out=out_pos, in_=acc2)

    nc.compile()
    return nc


_CACHE: dict = {}


def _get_nc(R_pad: int, nt2: int, nt2p: int):
    key = ("nc", R_pad, nt2, nt2p)
    if key not in _CACHE:
        _CACHE[key] = build_bass(R_pad, nt2, nt2p)
    return _CACHE[key]


def prepare_inputs(a: np.ndarray, b: np.ndarray, labels: np.ndarray):
    """Host-side label sort, class pairing, per-core shard + mask prep.

    Returns (per-core input maps, (R_pad, nt2, nt2p), meta)."""
    a = np.asarray(a, np.float32)
    b = np.asarray(b, np.float32)
    labels = np.asarray(labels)

    order = np.argsort(labels, kind="stable")
    a_s = a[order]
    b_s = b[order]
    sl = labels[order]
    counts = np.bincount(sl.astype(np.int64), minlength=NCLS)
    startscum = np.concatenate([[0], np.cumsum(counts)])

    def pair_tiles(p, q):
        ta = set(range(int(startscum[p]) // 128,
                       -(-int(startscum[p + 1]) // 128)))
        tb = set(range(int(startscum[q]) // 128,
                       -(-int(startscum[q + 1]) // 128)))
        return len(ta | tb)

    def pairing_cost(pairs_):
        return (max(int(counts[p] + counts[q]) for p, q in pairs_),
                max(pair_tiles(p, q) for p, q in pairs_))

    # greedy largest-with-smallest, then 2-opt swaps minimizing
    # (max pair size, max window tiles) lexicographically
    co = np.argsort(counts)
    pairs = [(int(co[i]), int(co[NCLS - 1 - i])) for i in range(NCORES)]
    best = pairing_cost(pairs)
    improved = True
    while improved:
        improved = False
        for i in range(NCORES):
            for j in range(i + 1, NCORES):
                for swap in ((0, 0), (0, 1)):
                    cand = list(pairs)
                    a1, b1 = pairs[i]
                    a2, b2 = pairs[j]
                    if swap == (0, 0):
                        cand[i], cand[j] = (a2, b1), (a1, b2)
                    else:
                        cand[i], cand[j] = (b2, b1), (a2, a1)
                    c = pairing_cost(cand)
                    if c < best:
                        pairs, best, improved = cand, c, True
    R_pad = best[0]
    R_pad = -(-R_pad // 32) * 32
    nU = -(-R_pad // 128)

    cores = []
    nt2 = 0
    for p, q in pairs:
        grows = np.concatenate([
            np.arange(startscum[p], startscum[p + 1]),
            np.arange(startscum[q], startscum[q + 1])])
        wtiles = sorted(set((grows // 128).tolist()))
        nt2 = max(nt2, len(wtiles))
        cores.append((p, q, grows, wtiles))
    nt2p = -(-nt2 // 4) * 4

    at_full = np.ascontiguousarray(a_s.T)                  # [F, N] sorted
    aa = np.sum(a_s * a_s, axis=1, dtype=np.float32)
    bb_s = np.sum(b_s * b_s, axis=1, dtype=np.float32)

    in_maps = []
    meta = []
    for c in range(NCORES):
        p, q, grows, wtiles = cores[c]
        Rc = len(grows)
        rest = [t for t in range(NT) if t not in wtiles]
        sigma = np.array(list(wtiles) + rest)
        slot_of = {t: s_ for s_, t in enumerate(sigma)}

        cols = (sigma[:, None] * 128 + np.arange(128)[None, :]).reshape(-1)
        # [128 kpart, 4 ksub, N] fp8 for DoubleRow lhsT slices
        at_c = np.ascontiguousarray(
            at_full[:, cols].reshape(4, 128, N).transpose(1, 0, 2)
        ).astype(NPFP8)
        aat_c = np.ascontiguousarray(aa[cols].reshape(NT, 128).T)  # [128, NT]

        glbl = sl[cols].reshape(NT, 128)                   # labels per slot
        oh3_c = np.zeros((NT, 128, 3), np.float32)
        oh3_c[:, :, 0] = 1.0
        oh3_c[:, :, 1] = glbl == p
        oh3_c[:, :, 2] = glbl == q
        oh3_c = np.ascontiguousarray(
            oh3_c.transpose(1, 0, 2).reshape(128, NT * 3)).astype(NPBF16)
        oh2_c = np.zeros((nt2, 128, 2), np.float32)
        oh2_c[:, :, 0] = glbl[:nt2] == p
        oh2_c[:, :, 1] = glbl[:nt2] == q
        oh2_c = np.ascontiguousarray(
            oh2_c.transpose(1, 0, 2).reshape(128, nt2 * 2)).astype(NPBF16)

        b_loc = np.zeros((R_pad, F), np.float32)
        b_loc[:Rc] = b_s[grows]
        a_my = np.zeros((R_pad, F), np.float32)
        a_my[:Rc] = a_s[grows]
        bb_loc = np.zeros(R_pad, np.float32)
        bb_loc[:Rc] = bb_s[grows]
        bt2_c = np.ascontiguousarray(
            (-2.0 * b_loc).T.reshape(4, 128, R_pad).transpose(1, 0, 2)
        ).astype(NPFP8)
        bbv_c = bb_loc.reshape(1, R_pad).copy()

        lbl_loc = np.full(R_pad, -1, np.int64)
        lbl_loc[:Rc] = sl[grows]
        selP = (lbl_loc == p).astype(np.float32)
        selQ = (lbl_loc == q).astype(np.float32)
        sel3_c = np.ascontiguousarray(
            np.stack([np.ones(R_pad, np.float32), selP, selQ], 0))
        ohmy2_c = np.ascontiguousarray(np.stack([selP, selQ], 0))

        ti_c = np.zeros((nU, 128, nt2p), np.float32)
        pm_c = np.zeros((nU + 1, 128, 128), np.float32)
        pm_c[nU] = np.eye(128, dtype=np.float32)
        used = np.zeros((128, nt2p), bool)
        for r in range(Rc):
            gr = grows[r]
            u, cc = r // 128, r % 128
            t_ = slot_of[gr // 128]
            ti_c[u, cc, t_] = 1.0
            pm_c[u, cc, gr % 128] = 1.0
            used[gr % 128, t_] = True
        ti_c = np.ascontiguousarray(
            ti_c.transpose(1, 0, 2).reshape(128, nU * nt2p)).astype(NPBF16)
        pm_c = np.ascontiguousarray(
            pm_c.transpose(1, 0, 2).reshape(128, (nU + 1) * 128)).astype(NPBF16)
        fill_c = np.where(used, 0.0, 1.0).astype(NPBF16)

        valid_c = (np.arange(R_pad) < Rc).astype(np.float32).reshape(1, R_pad)
        dd = np.sum(np.square(b_loc - a_my), axis=1, dtype=np.float32)
        ddiag_c = np.sqrt(np.maximum(dd, 0.0)).reshape(1, R_pad)

        in_maps.append({
            "at": at_c, "bt2": bt2_c, "aat": aat_c, "bbv": bbv_c,
            "oh3": oh3_c, "oh2": oh2_c, "sel3": sel3_c, "ohmy2": ohmy2_c,
            "ti": ti_c, "pm": pm_c, "fillm": np.ascontiguousarray(fill_c),
            "validm": valid_c, "ddiag": ddiag_c,
        })
        meta.append({"grows": grows, "Rc": Rc})
    return in_maps, (R_pad, nt2, nt2p), {"order": order, "cores": meta}


def run(a, b, labels, trace=False, trace_kwargs=None):
    """Run on 8 NeuronCores; returns (loss, BassKernelResults, meta)."""
    in_maps, dims, meta = prepare_inputs(a, b, labels)
    nc = _get_nc(*dims)
    kw = {}
    if trace:
        kw = dict(trace=True, **(trace_kwargs or {}))
    res = run_bass_kernel_spmd(nc, in_maps, core_ids=list(range(NCORES)), **kw)

    counts = np.bincount(np.asarray(labels).astype(np.int64), minlength=NCLS)
    num_pos = float((counts.astype(np.float64) ** 2).sum() - N)

    total = 0.0
    for c in range(NCORES):
        r = res.results[c]
        total += (float(r["out_pos"][0, 0]) + float(r["out_pos"][1, 0])
                  - float(r["out_diag"][0, 0]))
    loss = total / (2.0 * num_pos)
    return np.asarray(np.float32(loss)), res, meta


def kernel(a, b, labels):
    loss, _, _ = run(a, b, labels)
    return loss
